# revision 34
# baseline (speedup 1.0000x reference)
"""Bass/Trainium2 kernel for nn_CoreAttention (NTK causal attention with
linear phi-correction), SPMD over 8 NeuronCores.

Math (per batch b, head h; q,k,v: [n, d]; Z=phi_kv[h]: [d,d]; kk=|phi_k[h]|: [d,1]):
    phi_q  = ELU(q / d**0.25) + 1
    S      = q @ k.T / sqrt(d)
    A      = exp(S) * causal            # max-shift invariant -> use m=0
    num    = A @ v + phi_q @ Z
    den    = A @ ones + phi_q @ kk
    ctx    = num / den

Sharding: batch*head pairs (32) split 4-per-core across 8 cores. No
cross-core communication.  The final division num/den and the
[d, seq] -> [seq, d] transpose are done on the host (elementwise /
reshape glue).

Per-core structure (per pair):
    qtd  : [128, 2048] f16  -- q^T duplicated in both partition halves
    ktp  : [128, 8, 128] f16 -- k^T with even k-tiles in rows 0-63,
                                odd tiles in rows 64-127 (row-packed QK)
    vp   : [128, 16, 65] f16 -- V with ones column appended
    za   : [64, 65] f16      -- [Z | kk] * 2^7
    phiT : [64, 2048] f16    -- phi(q)^T * 2^-7, computed on-chip

Score tiles S^T [k,q] are produced by PAIRS of concurrent K=64 matmuls on
distinct PE row-groups (tile_position (0,0) / (64,0)) into a 2-bank PSUM
group.  exp() is split between ScalarE (table exp) and VectorE (Schraudolph
bit-trick: uint16(a*S + b) reinterpreted as f16) with a static greedy
load balance.  Causal diagonal blocks are masked by GpSimd (tril multiply).
ctx^T[65, 512] accumulates per (pair, q-block) in one PSUM bank via
matmuls with vp as the stationary operand (moving = exp tiles), opened by
the phi@[Z|kk] matmul.  A Scalar/Vector copy moves the bank to SBUF for
the output DMA.
"""

import math

import numpy as np

import concourse.bacc as bacc
import concourse.mybir as mybir
from concourse.tile import TileContext

SEQ, BS, H, D = 2048, 2, 16, 64
N_CORES = 8
NPAIR = BS * H            # 32 (b,h) pairs
PPC = NPAIR // N_CORES    # 4 pairs per core
P = 128                   # partition tile
NKT = SEQ // P            # 16 k tiles per pair
QB = 512                  # q-block width (one PSUM bank of fp32)
NQB = SEQ // QB           # 4 q blocks
DA = D + 1                # v augmented with ones column

_C = 1.0 / (D ** 0.25)        # phi input scale
_PHI_SCALE = 2.0 ** -7        # keep phi*Z product in fp16 normal range
_LN2_7 = 7.0 * math.log(2.0)
_EXP_SCALE = 1.0 / math.sqrt(D)

# Schraudolph fp16 exp: bits = trunc(A*S + B); calibrated so the mean
# approx/exact ratio is 1.0 (mixing with exact ScalarE exp tiles).
_SCH_A = 1024.0 * math.log2(math.e) * _EXP_SCALE
_SCH_B = 1024.0 * 15.0 - 59.11

# Set by test harness only; grading path uses defaults.
TRACE = False
LAST_RESULT = None

_cached_nc = None


def _plan_schedule():
    """Static per-pair schedule: for each (qb, g) group, the ScalarE/VectorE
    assignment for exp groups and output copies, balanced with HW-measured
    per-instruction rates (ns)."""
    s_tot = 0.0
    v_tot = 0.0
    assign = {}
    for qb in range(NQB):
        for g in range(2 * (qb + 1)):
            t0 = 2 * g - 4 * qb
            t1 = t0 + 1
            nmask = (t0 >= 0) + (t1 >= 0)
            if t1 <= 1:
                cs, cv = 750.0, 1080.0
            else:
                cs, cv = 610.0, 900.0
            # V-produced tiles mask their own diagonal blocks on the DVE
            cv += 130.0 * nmask
            if s_tot + cs <= v_tot + cv:
                assign[(qb, g)] = "S"
                s_tot += cs
            else:
                assign[(qb, g)] = "V"
                v_tot += cv
        cs, cv = 570.0, 684.0
        if s_tot + cs <= v_tot + cv:
            assign[("copy", qb)] = "S"
            s_tot += cs
        else:
            assign[("copy", qb)] = "V"
            v_tot += cv
    return assign


def _build_module():
    f16 = mybir.dt.float16
    f32 = mybir.dt.float32
    u16 = mybir.dt.uint16
    Exp = mybir.ActivationFunctionType.Exp
    Alu = mybir.AluOpType

    assign = _plan_schedule()

    nc = bacc.Bacc("TRN2", target_bir_lowering=False, debug=False)

    d_qtd = nc.dram_tensor("qtd", [PPC, P, SEQ], f16, kind="ExternalInput")
    d_ktp = nc.dram_tensor("ktp", [PPC, P, NKT // 2, P], f16, kind="ExternalInput")
    d_vp = nc.dram_tensor("vp", [PPC, P, NKT, DA], f16, kind="ExternalInput")
    d_za = nc.dram_tensor("za", [PPC, D, DA], f16, kind="ExternalInput")
    d_phi = nc.dram_tensor("phiT", [PPC, D, SEQ], f16, kind="ExternalInput")
    d_tril = nc.dram_tensor("tril", [P, P], f16, kind="ExternalInput")
    d_out = nc.dram_tensor("out", [PPC, NQB, DA, QB], f32, kind="ExternalOutput")

    with TileContext(nc) as tc:
        with (
            tc.tile_pool(name="const", bufs=1) as constp,
            tc.tile_pool(name="pairbuf", bufs=3) as pairp,
            tc.tile_pool(name="exbuf", bufs=6) as exp_pool,
            tc.tile_pool(name="scps", bufs=3, space="PSUM") as scp,
            tc.tile_pool(name="avps", bufs=2, space="PSUM") as avp,
            tc.tile_pool(name="outbuf", bufs=4) as outp,
        ):
            tril_t = constp.tile([P, P], f16)
            nc.sync.dma_start(out=tril_t, in_=d_tril[:, :])

            # PE clock warm-up: the HAM un-throttles (1.2 -> 2.4 GHz) only
            # after a fully-busy activity window. A dense dep-free burst here
            # bridges until the first input DMAs land; the main loop's
            # back-to-back matmuls then keep the busy window alive.
            warm_in = constp.tile([P, QB], f16)
            nc.vector.memset(warm_in, 0.0)
            wsc = scp.tile([P, 2, QB], f32, tag="sc")
            for w in range(8):
                nc.tensor.matmul(
                    out=wsc[:, w % 2, :],
                    lhsT=warm_in[:, (w % 4) * P: (w % 4 + 1) * P],
                    rhs=warm_in,
                    start=True, stop=True,
                )

            pair_tiles = {}
            av_tiles = {}

            def load_pair(pair):
                if pair in pair_tiles or pair >= PPC:
                    return
                qtd = pairp.tile([P, SEQ], f16, tag="qtd")
                ktp = pairp.tile([P, NKT // 2, P], f16, tag="ktp")
                vp = pairp.tile([P, NKT, DA], f16, tag="vp")
                za = pairp.tile([D, DA], f16, tag="za")
                phiT = pairp.tile([D, SEQ], f16, tag="phiT")
                nc.sync.dma_start(out=qtd[:, 0:QB], in_=d_qtd[pair, :, 0:QB])
                nc.sync.dma_start(out=ktp, in_=d_ktp[pair])
                nc.sync.dma_start(out=phiT[:, 0:QB], in_=d_phi[pair, :, 0:QB])
                nc.sync.dma_start(out=vp, in_=d_vp[pair])
                nc.sync.dma_start(out=za, in_=d_za[pair])
                for ch in range(1, NQB):
                    s = slice(ch * QB, (ch + 1) * QB)
                    nc.sync.dma_start(out=qtd[:, s], in_=d_qtd[pair, :, s])
                    nc.sync.dma_start(out=phiT[:, s], in_=d_phi[pair, :, s])
                pair_tiles[pair] = (qtd, ktp, vp, za, phiT)

            def emit_qk(step):
                pair, qb, g = step
                if qb == 0 and g == 0:
                    load_pair(pair)
                qtd, ktp, vp, za, phiT = pair_tiles[pair]
                q0 = qb * QB
                sc = scp.tile([P, 2, QB], f32, tag="sc")
                for u in range(2):
                    t = 2 * g + u - 4 * qb
                    c0 = t * P if t >= 1 else 0
                    nc.tensor.matmul(
                        out=sc[:, u, c0:QB],
                        lhsT=ktp[u * D:(u + 1) * D, g, :],
                        rhs=qtd[u * D: u * D + D, q0 + c0: q0 + QB],
                        start=True, stop=True,
                        tile_position=(u * D, 0),
                    )
                if g == 0:
                    # open the ctx^T accumulation bank: [Z|kk]^T @ phi^T.
                    # Emitted in the row-group phase (h0) so it overlaps the
                    # h64 QK matmul instead of costing a full-row transition.
                    av = avp.tile([P, QB], f32, tag="av")
                    av_tiles[(pair, qb)] = av
                    nc.tensor.matmul(
                        out=av[0:DA, :],
                        lhsT=za,
                        rhs=phiT[:, q0: q0 + QB],
                        start=True, stop=False,
                        tile_position=(0, 0),
                    )
                return sc

            # last pair runs its q-blocks largest-first so the pipeline
            # drains on the shortest block (smaller idle tail)
            steps = []
            for pair in range(PPC):
                qb_order = (range(NQB) if pair < PPC - 1
                            else reversed(range(NQB)))
                for qb in qb_order:
                    for g in range(2 * (qb + 1)):
                        steps.append((pair, qb, g))
            sc_tiles = {}
            for i in range(min(2, len(steps))):
                sc_tiles[i] = emit_qk(steps[i])

            for i, step in enumerate(steps):
                # batch QK emission three groups at a time so the PE pays the
                # row-group <-> full-row transition once per three groups
                if i % 3 == 0:
                    for i2 in (i + 2, i + 3, i + 4):
                        if i2 < len(steps):
                            sc_tiles[i2] = emit_qk(steps[i2])
                pair, qb, g = step
                if qb == 1 and g == 0:
                    load_pair(pair + 1)   # prefetch next pair early
                qtd, ktp, vp, za, phiT = pair_tiles[pair]
                q0 = qb * QB
                sc = sc_tiles.pop(i)
                n_groups = 2 * (qb + 1)
                ts = [2 * g - 4 * qb, 2 * g + 1 - 4 * qb]
                eng = assign[(qb, g)]

                av = av_tiles[(pair, qb)]

                # exp (ScalarE table exp / VectorE Schraudolph), PSUM->SBUF f16
                ex = exp_pool.tile([P, 2, QB], f16, tag="ex")
                if ts[1] <= 1:
                    if eng == "S":
                        nc.scalar.activation(
                            out=ex[:, :, :], in_=sc[:, :, :],
                            func=Exp, scale=_EXP_SCALE,
                        )
                    else:
                        nc.vector.tensor_scalar(
                            out=ex[:, :, :].bitcast(u16), in0=sc[:, :, :],
                            scalar1=_SCH_A, scalar2=_SCH_B,
                            op0=Alu.mult, op1=Alu.add,
                        )
                else:
                    for u in range(2):
                        c0 = ts[u] * P
                        if eng == "S":
                            nc.scalar.activation(
                                out=ex[:, u, c0:QB], in_=sc[:, u, c0:QB],
                                func=Exp, scale=_EXP_SCALE,
                            )
                        else:
                            nc.vector.tensor_scalar(
                                out=ex[:, u, c0:QB].bitcast(u16),
                                in0=sc[:, u, c0:QB],
                                scalar1=_SCH_A, scalar2=_SCH_B,
                                op0=Alu.mult, op1=Alu.add,
                            )
                # causal diagonal blocks: tril mask — on the DVE for its own
                # tiles (no cross-engine wait in the DVE FIFO), on GpSimd for
                # ScalarE tiles (idle engine, stalls are harmless there)
                for u in range(2):
                    t = ts[u]
                    if t >= 0:
                        if eng == "V":
                            nc.vector.tensor_mul(
                                out=ex[:, u, t * P:(t + 1) * P],
                                in0=ex[:, u, t * P:(t + 1) * P],
                                in1=tril_t,
                            )
                        else:
                            nc.gpsimd.tensor_tensor(
                                out=ex[:, u, t * P:(t + 1) * P],
                                in0=ex[:, u, t * P:(t + 1) * P],
                                in1=tril_t,
                                op=Alu.mult,
                            )

                # ctx^T += vp_j^T @ ex_j  (vp stationary, ex moving)
                for u in range(2):
                    j = 2 * g + u
                    t = ts[u]
                    c0 = t * P if t >= 1 else 0
                    last = (g == n_groups - 1 and u == 1)
                    nc.tensor.matmul(
                        out=av[0:DA, c0:QB],
                        lhsT=vp[:, j, :],
                        rhs=ex[:, u, c0:QB],
                        start=False, stop=last,
                    )

                if g == n_groups - 1:
                    av_tiles.pop((pair, qb))
                    out_t = outp.tile([DA, QB], f32, tag="out_t")
                    if assign[("copy", qb)] == "S":
                        nc.scalar.copy(out=out_t, in_=av[0:DA, :])
                    else:
                        nc.vector.tensor_copy(out=out_t, in_=av[0:DA, :])
                    nc.sync.dma_start(out=d_out[pair, qb], in_=out_t)

    nc.compile()
    return nc


def _prep_core_inputs(query_layer, key_layer, value_layer, phi_k, phi_kv):
    q = np.asarray(query_layer, dtype=np.float32)
    k = np.asarray(key_layer, dtype=np.float32)
    v = np.asarray(value_layer, dtype=np.float32)
    zk = np.abs(np.asarray(phi_k, dtype=np.float32))[0, :, :, 0]   # [H, D]
    zv = np.asarray(phi_kv, dtype=np.float32)[0]                   # [H, D, D]

    # [seq,bs,h,d] -> per-pair transposed [pair, d, seq]
    qT = np.ascontiguousarray(q.transpose(1, 2, 3, 0).reshape(NPAIR, D, SEQ))
    kT = np.ascontiguousarray(k.transpose(1, 2, 3, 0).reshape(NPAIR, D, SEQ))

    # q^T duplicated into both partition halves for row-packed matmuls
    qtd = np.concatenate([qT, qT], axis=1)                         # [pair, 128, seq]

    # k^T packed: even k-tiles in rows 0-63, odd in rows 64-127
    ktt = kT.reshape(NPAIR, D, NKT // 2, 2, P)                     # [pair, d, g, 2, p]
    ktp = np.concatenate([ktt[:, :, :, 0, :], ktt[:, :, :, 1, :]],
                         axis=1)                                   # [pair, 128, g, p]

    vn = v.transpose(1, 2, 0, 3).reshape(NPAIR, SEQ, D)            # [pair, n, d]
    v_aug = np.concatenate(
        [vn, np.ones((NPAIR, SEQ, 1), np.float32)], axis=2)        # [pair, n, 65]
    vp = np.ascontiguousarray(
        v_aug.reshape(NPAIR, NKT, P, DA).transpose(0, 2, 1, 3))    # [pair, p, j, 65]

    za_h = np.concatenate([zv, zk[:, :, None]], axis=2) / _PHI_SCALE  # [H, D, 65]
    za = za_h[np.arange(NPAIR) % H]                                # [pair, d, 65]

    # phi(q)^T * 2^-7 precomputed on host (elementwise input prep)
    y = qT * _C                                                    # [pair, d, seq]
    phiT = (np.exp(np.minimum(y, 0.0)) + np.maximum(y, 0.0)) * _PHI_SCALE

    tril = np.triu(np.ones((P, P), np.float32))                    # keep k<=q in S^T

    in_maps = []
    for c in range(N_CORES):
        s = slice(c * PPC, (c + 1) * PPC)
        in_maps.append({
            "qtd": qtd[s].astype(np.float16),
            "ktp": np.ascontiguousarray(ktp[s]).astype(np.float16),
            "vp": vp[s].astype(np.float16),
            "za": za[s].astype(np.float16),
            "phiT": phiT[s].astype(np.float16),
            "tril": tril.astype(np.float16),
        })
    return in_maps


def _install_trace_shim():
    import sys
    import types
    if "antenv.axon_hooks" not in sys.modules:
        m = types.ModuleType("antenv.axon_hooks")
        m._hook = None
        m.set_axon_ntff_profile_hook = lambda h: setattr(m, "_hook", h)
        m.get_axon_ntff_profile_hook = lambda: m._hook
        sys.modules["antenv.axon_hooks"] = m
        import antenv
        antenv.axon_hooks = m
    from trn_agent_boot.trn_boot import _ntff_profile_via_ctypes
    sys.modules["antenv.axon_hooks"].set_axon_ntff_profile_hook(
        _ntff_profile_via_ctypes("/opt/axon/libaxon_pjrt.so"))
    import concourse.bass_utils as bu
    bu.upload_artifacts = lambda tmpdir: "local://" + str(tmpdir)


def kernel(query_layer, key_layer, value_layer, attention_mask, phi_k, phi_kv):
    global _cached_nc, LAST_RESULT
    from concourse.bass_utils import run_bass_kernel_spmd

    if TRACE:
        _install_trace_shim()
    if _cached_nc is None:
        _cached_nc = _build_module()
    nc = _cached_nc

    in_maps = _prep_core_inputs(
        query_layer, key_layer, value_layer, phi_k, phi_kv)
    res = run_bass_kernel_spmd(
        nc, in_maps, core_ids=list(range(N_CORES)), trace=TRACE)
    LAST_RESULT = res

    outs = np.stack([res.results[c]["out"] for c in range(N_CORES)])
    outs = outs.reshape(NPAIR, NQB, DA, QB)
    num = outs[:, :, 0:D, :]                                 # [pair, qb, d, col]
    den = outs[:, :, D, :]                                   # [pair, qb, col]
    ctxT = num / den[:, :, None, :]
    ctx = ctxT.transpose(0, 1, 3, 2).reshape(NPAIR, SEQ, D)  # [pair, q, d]
    ctx = ctx.reshape(BS, H, SEQ, D).transpose(2, 0, 1, 3)   # [n,bs,h,d]
    return np.ascontiguousarray(ctx.reshape(SEQ, BS, H * D)).astype(np.float32)


# revision 35
# speedup vs baseline: 1.0131x; 1.0131x over previous
"""Bass/Trainium2 kernel for nn_CoreAttention (NTK causal attention with
linear phi-correction), SPMD over 8 NeuronCores.

Math (per batch b, head h; q,k,v: [n, d]; Z=phi_kv[h]: [d,d]; kk=|phi_k[h]|: [d,1]):
    phi_q  = ELU(q / d**0.25) + 1
    S      = q @ k.T / sqrt(d)
    A      = exp(S) * causal            # max-shift invariant -> use m=0
    num    = A @ v + phi_q @ Z
    den    = A @ ones + phi_q @ kk
    ctx    = num / den

Sharding: batch*head pairs (32) split 4-per-core across 8 cores. No
cross-core communication.  The final division num/den and the
[d, seq] -> [seq, d] transpose are done on the host (elementwise /
reshape glue).

Per-core structure (per pair):
    qtd  : [128, 2048] f16  -- q^T duplicated in both partition halves
    ktp  : [128, 8, 128] f16 -- k^T with even k-tiles in rows 0-63,
                                odd tiles in rows 64-127 (row-packed QK)
    vp   : [128, 16, 65] f16 -- V with ones column appended
    za   : [64, 65] f16      -- [Z | kk] * 2^7
    phiT : [64, 2048] f16    -- phi(q)^T * 2^-7, computed on-chip

Score tiles S^T [k,q] are produced by PAIRS of concurrent K=64 matmuls on
distinct PE row-groups (tile_position (0,0) / (64,0)) into a 2-bank PSUM
group.  exp() is split between ScalarE (table exp) and VectorE (Schraudolph
bit-trick: uint16(a*S + b) reinterpreted as f16) with a static greedy
load balance.  Causal diagonal blocks are masked by GpSimd (tril multiply).
ctx^T[65, 512] accumulates per (pair, q-block) in one PSUM bank via
matmuls with vp as the stationary operand (moving = exp tiles), opened by
the phi@[Z|kk] matmul.  A Scalar/Vector copy moves the bank to SBUF for
the output DMA.
"""

import math

import numpy as np

import concourse.bacc as bacc
import concourse.mybir as mybir
from concourse.tile import TileContext

SEQ, BS, H, D = 2048, 2, 16, 64
N_CORES = 8
NPAIR = BS * H            # 32 (b,h) pairs
PPC = NPAIR // N_CORES    # 4 pairs per core
P = 128                   # partition tile
NKT = SEQ // P            # 16 k tiles per pair
QB = 512                  # q-block width (one PSUM bank of fp32)
NQB = SEQ // QB           # 4 q blocks
DA = D + 1                # v augmented with ones column

_C = 1.0 / (D ** 0.25)        # phi input scale
_PHI_SCALE = 2.0 ** -7        # keep phi*Z product in fp16 normal range
_LN2_7 = 7.0 * math.log(2.0)
_EXP_SCALE = 1.0 / math.sqrt(D)

# Schraudolph fp16 exp: bits = trunc(A*S + B); calibrated so the mean
# approx/exact ratio is 1.0 (mixing with exact ScalarE exp tiles).
_SCH_A = 1024.0 * math.log2(math.e) * _EXP_SCALE
_SCH_B = 1024.0 * 15.0 - 59.11

# Set by test harness only; grading path uses defaults.
TRACE = False
LAST_RESULT = None

_cached_nc = None


def _plan_schedule():
    """Static per-pair schedule: for each (qb, g) group, the ScalarE/VectorE
    assignment for exp groups and output copies, balanced with HW-measured
    per-instruction rates (ns)."""
    s_tot = 0.0
    v_tot = 0.0
    assign = {}
    for qb in range(NQB):
        for g in range(2 * (qb + 1)):
            t0 = 2 * g - 4 * qb
            t1 = t0 + 1
            nmask = (t0 >= 0) + (t1 >= 0)
            if t1 <= 1:
                cs, cv = 750.0, 1080.0
            else:
                cs, cv = 610.0, 900.0
            # V-produced tiles mask their own diagonal blocks on the DVE
            cv += 130.0 * nmask
            if s_tot + cs <= v_tot + cv:
                assign[(qb, g)] = "S"
                s_tot += cs
            else:
                assign[(qb, g)] = "V"
                v_tot += cv
        cs, cv = 570.0, 684.0
        if s_tot + cs <= v_tot + cv:
            assign[("copy", qb)] = "S"
            s_tot += cs
        else:
            assign[("copy", qb)] = "V"
            v_tot += cv
    return assign


def _build_module():
    f16 = mybir.dt.float16
    f32 = mybir.dt.float32
    u16 = mybir.dt.uint16
    Exp = mybir.ActivationFunctionType.Exp
    Alu = mybir.AluOpType

    assign = _plan_schedule()

    nc = bacc.Bacc("TRN2", target_bir_lowering=False, debug=False)

    d_qtd = nc.dram_tensor("qtd", [PPC, P, SEQ], f16, kind="ExternalInput")
    d_ktp = nc.dram_tensor("ktp", [PPC, P, NKT // 2, P], f16, kind="ExternalInput")
    d_vp = nc.dram_tensor("vp", [PPC, P, NKT, DA], f16, kind="ExternalInput")
    d_za = nc.dram_tensor("za", [PPC, D, DA], f16, kind="ExternalInput")
    d_phi = nc.dram_tensor("phiT", [PPC, D, SEQ], f16, kind="ExternalInput")
    d_tril = nc.dram_tensor("tril", [P, P], f16, kind="ExternalInput")
    d_out = nc.dram_tensor("out", [PPC, NQB, DA, QB], f32, kind="ExternalOutput")

    with TileContext(nc) as tc:
        with (
            tc.tile_pool(name="const", bufs=1) as constp,
            tc.tile_pool(name="pairbuf", bufs=3) as pairp,
            tc.tile_pool(name="exbuf", bufs=6) as exp_pool,
            tc.tile_pool(name="scps", bufs=3, space="PSUM") as scp,
            tc.tile_pool(name="avps", bufs=2, space="PSUM") as avp,
            tc.tile_pool(name="outbuf", bufs=4) as outp,
        ):
            tril_t = constp.tile([P, P], f16)
            nc.sync.dma_start(out=tril_t, in_=d_tril[:, :])

            # PE clock warm-up: the HAM un-throttles (1.2 -> 2.4 GHz) only
            # after a fully-busy activity window, which the dependency-laced
            # early main loop never produces from cold (tested: a shorter
            # burst lets the HAM re-throttle during ramp-up, costing more
            # than the head time it saves). A dense dep-free burst here
            # also bridges until the first input DMAs land.
            warm_in = constp.tile([P, QB], f16)
            nc.vector.memset(warm_in, 0.0)
            wsc = scp.tile([P, 2, QB], f32, tag="sc")
            for w in range(18):
                nc.tensor.matmul(
                    out=wsc[:, w % 2, :],
                    lhsT=warm_in[:, (w % 4) * P: (w % 4 + 1) * P],
                    rhs=warm_in,
                    start=True, stop=True,
                )

            pair_tiles = {}
            av_tiles = {}

            def load_pair(pair):
                if pair in pair_tiles or pair >= PPC:
                    return
                qtd = pairp.tile([P, SEQ], f16, tag="qtd")
                ktp = pairp.tile([P, NKT // 2, P], f16, tag="ktp")
                vp = pairp.tile([P, NKT, DA], f16, tag="vp")
                za = pairp.tile([D, DA], f16, tag="za")
                phiT = pairp.tile([D, SEQ], f16, tag="phiT")
                nc.sync.dma_start(out=qtd[:, 0:QB], in_=d_qtd[pair, :, 0:QB])
                nc.sync.dma_start(out=ktp, in_=d_ktp[pair])
                nc.sync.dma_start(out=phiT[:, 0:QB], in_=d_phi[pair, :, 0:QB])
                nc.sync.dma_start(out=vp, in_=d_vp[pair])
                nc.sync.dma_start(out=za, in_=d_za[pair])
                for ch in range(1, NQB):
                    s = slice(ch * QB, (ch + 1) * QB)
                    nc.sync.dma_start(out=qtd[:, s], in_=d_qtd[pair, :, s])
                    nc.sync.dma_start(out=phiT[:, s], in_=d_phi[pair, :, s])
                pair_tiles[pair] = (qtd, ktp, vp, za, phiT)

            def emit_qk(step):
                pair, qb, g = step
                if qb == 0 and g == 0:
                    load_pair(pair)
                qtd, ktp, vp, za, phiT = pair_tiles[pair]
                q0 = qb * QB
                sc = scp.tile([P, 2, QB], f32, tag="sc")
                for u in range(2):
                    t = 2 * g + u - 4 * qb
                    c0 = t * P if t >= 1 else 0
                    nc.tensor.matmul(
                        out=sc[:, u, c0:QB],
                        lhsT=ktp[u * D:(u + 1) * D, g, :],
                        rhs=qtd[u * D: u * D + D, q0 + c0: q0 + QB],
                        start=True, stop=True,
                        tile_position=(u * D, 0),
                    )
                if g == 0:
                    # open the ctx^T accumulation bank: [Z|kk]^T @ phi^T.
                    # Emitted in the row-group phase (h0) so it overlaps the
                    # h64 QK matmul instead of costing a full-row transition.
                    av = avp.tile([P, QB], f32, tag="av")
                    av_tiles[(pair, qb)] = av
                    nc.tensor.matmul(
                        out=av[0:DA, :],
                        lhsT=za,
                        rhs=phiT[:, q0: q0 + QB],
                        start=True, stop=False,
                        tile_position=(0, 0),
                    )
                return sc

            # last pair runs its q-blocks largest-first so the pipeline
            # drains on the shortest block (smaller idle tail)
            steps = []
            for pair in range(PPC):
                qb_order = (range(NQB) if pair < PPC - 1
                            else reversed(range(NQB)))
                for qb in qb_order:
                    for g in range(2 * (qb + 1)):
                        steps.append((pair, qb, g))
            sc_tiles = {}
            for i in range(min(2, len(steps))):
                sc_tiles[i] = emit_qk(steps[i])

            for i, step in enumerate(steps):
                # batch QK emission three groups at a time so the PE pays the
                # row-group <-> full-row transition once per three groups
                if i % 3 == 0:
                    for i2 in (i + 2, i + 3, i + 4):
                        if i2 < len(steps):
                            sc_tiles[i2] = emit_qk(steps[i2])
                pair, qb, g = step
                if qb == 1 and g == 0:
                    load_pair(pair + 1)   # prefetch next pair early
                qtd, ktp, vp, za, phiT = pair_tiles[pair]
                q0 = qb * QB
                sc = sc_tiles.pop(i)
                n_groups = 2 * (qb + 1)
                ts = [2 * g - 4 * qb, 2 * g + 1 - 4 * qb]
                eng = assign[(qb, g)]

                av = av_tiles[(pair, qb)]

                # exp (ScalarE table exp / VectorE Schraudolph), PSUM->SBUF f16
                ex = exp_pool.tile([P, 2, QB], f16, tag="ex")
                if ts[1] <= 1:
                    if eng == "S":
                        nc.scalar.activation(
                            out=ex[:, :, :], in_=sc[:, :, :],
                            func=Exp, scale=_EXP_SCALE,
                        )
                    else:
                        nc.vector.tensor_scalar(
                            out=ex[:, :, :].bitcast(u16), in0=sc[:, :, :],
                            scalar1=_SCH_A, scalar2=_SCH_B,
                            op0=Alu.mult, op1=Alu.add,
                        )
                else:
                    for u in range(2):
                        c0 = ts[u] * P
                        if eng == "S":
                            nc.scalar.activation(
                                out=ex[:, u, c0:QB], in_=sc[:, u, c0:QB],
                                func=Exp, scale=_EXP_SCALE,
                            )
                        else:
                            nc.vector.tensor_scalar(
                                out=ex[:, u, c0:QB].bitcast(u16),
                                in0=sc[:, u, c0:QB],
                                scalar1=_SCH_A, scalar2=_SCH_B,
                                op0=Alu.mult, op1=Alu.add,
                            )
                # causal diagonal blocks: tril mask — on the DVE for its own
                # tiles (no cross-engine wait in the DVE FIFO), on GpSimd for
                # ScalarE tiles (idle engine, stalls are harmless there)
                for u in range(2):
                    t = ts[u]
                    if t >= 0:
                        if eng == "V":
                            nc.vector.tensor_mul(
                                out=ex[:, u, t * P:(t + 1) * P],
                                in0=ex[:, u, t * P:(t + 1) * P],
                                in1=tril_t,
                            )
                        else:
                            nc.gpsimd.tensor_tensor(
                                out=ex[:, u, t * P:(t + 1) * P],
                                in0=ex[:, u, t * P:(t + 1) * P],
                                in1=tril_t,
                                op=Alu.mult,
                            )

                # ctx^T += vp_j^T @ ex_j  (vp stationary, ex moving)
                for u in range(2):
                    j = 2 * g + u
                    t = ts[u]
                    c0 = t * P if t >= 1 else 0
                    last = (g == n_groups - 1 and u == 1)
                    nc.tensor.matmul(
                        out=av[0:DA, c0:QB],
                        lhsT=vp[:, j, :],
                        rhs=ex[:, u, c0:QB],
                        start=False, stop=last,
                    )

                if g == n_groups - 1:
                    av_tiles.pop((pair, qb))
                    out_t = outp.tile([DA, QB], f32, tag="out_t")
                    if assign[("copy", qb)] == "S":
                        nc.scalar.copy(out=out_t, in_=av[0:DA, :])
                    else:
                        nc.vector.tensor_copy(out=out_t, in_=av[0:DA, :])
                    nc.sync.dma_start(out=d_out[pair, qb], in_=out_t)

    nc.compile()
    return nc


def _prep_core_inputs(query_layer, key_layer, value_layer, phi_k, phi_kv):
    q = np.asarray(query_layer, dtype=np.float32)
    k = np.asarray(key_layer, dtype=np.float32)
    v = np.asarray(value_layer, dtype=np.float32)
    zk = np.abs(np.asarray(phi_k, dtype=np.float32))[0, :, :, 0]   # [H, D]
    zv = np.asarray(phi_kv, dtype=np.float32)[0]                   # [H, D, D]

    # [seq,bs,h,d] -> per-pair transposed [pair, d, seq]
    qT = np.ascontiguousarray(q.transpose(1, 2, 3, 0).reshape(NPAIR, D, SEQ))
    kT = np.ascontiguousarray(k.transpose(1, 2, 3, 0).reshape(NPAIR, D, SEQ))

    # q^T duplicated into both partition halves for row-packed matmuls
    qtd = np.concatenate([qT, qT], axis=1)                         # [pair, 128, seq]

    # k^T packed: even k-tiles in rows 0-63, odd in rows 64-127
    ktt = kT.reshape(NPAIR, D, NKT // 2, 2, P)                     # [pair, d, g, 2, p]
    ktp = np.concatenate([ktt[:, :, :, 0, :], ktt[:, :, :, 1, :]],
                         axis=1)                                   # [pair, 128, g, p]

    vn = v.transpose(1, 2, 0, 3).reshape(NPAIR, SEQ, D)            # [pair, n, d]
    v_aug = np.concatenate(
        [vn, np.ones((NPAIR, SEQ, 1), np.float32)], axis=2)        # [pair, n, 65]
    vp = np.ascontiguousarray(
        v_aug.reshape(NPAIR, NKT, P, DA).transpose(0, 2, 1, 3))    # [pair, p, j, 65]

    za_h = np.concatenate([zv, zk[:, :, None]], axis=2) / _PHI_SCALE  # [H, D, 65]
    za = za_h[np.arange(NPAIR) % H]                                # [pair, d, 65]

    # phi(q)^T * 2^-7 precomputed on host (elementwise input prep)
    y = qT * _C                                                    # [pair, d, seq]
    phiT = (np.exp(np.minimum(y, 0.0)) + np.maximum(y, 0.0)) * _PHI_SCALE

    tril = np.triu(np.ones((P, P), np.float32))                    # keep k<=q in S^T

    in_maps = []
    for c in range(N_CORES):
        s = slice(c * PPC, (c + 1) * PPC)
        in_maps.append({
            "qtd": qtd[s].astype(np.float16),
            "ktp": np.ascontiguousarray(ktp[s]).astype(np.float16),
            "vp": vp[s].astype(np.float16),
            "za": za[s].astype(np.float16),
            "phiT": phiT[s].astype(np.float16),
            "tril": tril.astype(np.float16),
        })
    return in_maps


def _install_trace_shim():
    import sys
    import types
    if "antenv.axon_hooks" not in sys.modules:
        m = types.ModuleType("antenv.axon_hooks")
        m._hook = None
        m.set_axon_ntff_profile_hook = lambda h: setattr(m, "_hook", h)
        m.get_axon_ntff_profile_hook = lambda: m._hook
        sys.modules["antenv.axon_hooks"] = m
        import antenv
        antenv.axon_hooks = m
    from trn_agent_boot.trn_boot import _ntff_profile_via_ctypes
    sys.modules["antenv.axon_hooks"].set_axon_ntff_profile_hook(
        _ntff_profile_via_ctypes("/opt/axon/libaxon_pjrt.so"))
    import concourse.bass_utils as bu
    bu.upload_artifacts = lambda tmpdir: "local://" + str(tmpdir)


def kernel(query_layer, key_layer, value_layer, attention_mask, phi_k, phi_kv):
    global _cached_nc, LAST_RESULT
    from concourse.bass_utils import run_bass_kernel_spmd

    if TRACE:
        _install_trace_shim()
    if _cached_nc is None:
        _cached_nc = _build_module()
    nc = _cached_nc

    in_maps = _prep_core_inputs(
        query_layer, key_layer, value_layer, phi_k, phi_kv)
    res = run_bass_kernel_spmd(
        nc, in_maps, core_ids=list(range(N_CORES)), trace=TRACE)
    LAST_RESULT = res

    outs = np.stack([res.results[c]["out"] for c in range(N_CORES)])
    outs = outs.reshape(NPAIR, NQB, DA, QB)
    num = outs[:, :, 0:D, :]                                 # [pair, qb, d, col]
    den = outs[:, :, D, :]                                   # [pair, qb, col]
    ctxT = num / den[:, :, None, :]
    ctx = ctxT.transpose(0, 1, 3, 2).reshape(NPAIR, SEQ, D)  # [pair, q, d]
    ctx = ctx.reshape(BS, H, SEQ, D).transpose(2, 0, 1, 3)   # [n,bs,h,d]
    return np.ascontiguousarray(ctx.reshape(SEQ, BS, H * D)).astype(np.float32)
